# revision 14
# baseline (speedup 1.0000x reference)
"""Trainium2 Bass kernel for nn_BottleneckFFN.

Computes y = LayerNorm(GELU(x @ W1.T + b1) @ W2.T + b2) * gamma + beta
for x of shape (128, 2048, 256), W1 (8, 256), W2 (8, 8), LN over the
trailing 8 channels.  Pure data parallel over 8 NeuronCores: the
128*2048 = 262144 token rows are split into 8 shards of 32768 tokens;
the tiny weights are replicated.

Per-core dataflow (per round of 2048 tokens):
  1. DMA 2 MB of x rows into SBUF, token-major ([128 part, 16 tiles, 256]).
  2. DVE 32x32 block-transposes flip each tile to feature-major per
     32-partition group, downcasting to bf16 on the write (no PE
     transpose, no PSUM round trip).  bf16 matmuls keep full PE speed
     (1 cycle/row) without float32r's psum-partition-base-0 restriction,
     so everything below uses all 128 partitions.  Measured host-side:
     the bf16 pipeline lands at 3.5e-3 rel err vs the fp32 reference.
  3. Two half-rounds of 1024 tokens, double-buffered in PSUM so
     mm1(h+1) overlaps GELU(h):
     a. mm1: 8 d-blocks x 4 concurrent diagonal K=32 bf16 matmuls
        (tile_position (32P, 32P)) accumulate x @ W1.T into ONE psum
        bank as [128, 256]: token group P's channels at partitions
        32P..32P+32, the proven-safe same-bank different-partition
        pattern.
     b. Exact GELU over all 128 lanes (256 cycles, 8x fewer than the
        old 32-partition layout), b1 fused as per-partition bias,
        bf16 output feeding mm2 directly.
     c. mm2: 4 concurrent diagonal K=8 bf16 matmuls with a 32-col
        stationary whose col 8 is mean(W2 rows), so the per-token LN
        mean falls out of the matmul; fresh double-buffered psum bank.
     d. One DVE block-transpose back to token-major.
     e. centered = h2 - mu (GpSimd), Square (GpSimd), grouped reduce
        (DVE) into per-batch accumulators.
  4. After rounds 0-7 and again after 8-15: one Sqrt (ACT, amortizing
     the Gelu<->Sqrt table switches) + DVE reciprocal gives rstd;
     per-round GpSimd scale + DMA out issued from GpSimd (never from
     ScalarE, whose in-order stream would stall GELU behind ring-full
     dma_starts while the x loads saturate HBM).
"""

import os
import sys

import numpy as np

if not any(os.path.isdir(os.path.join(p, "concourse")) for p in sys.path if p):
    for _cand in ("/opt/trn_rl_repo", "/root/.axon_site/_ro/trn_rl_repo"):
        if os.path.isdir(os.path.join(_cand, "concourse")):
            sys.path.insert(0, _cand)
            break

N_CORES = 8
DIM, OUT = 256, 8
B, T = 128, 2048
TOK_TOTAL = B * T
TOK_CORE = TOK_TOTAL // N_CORES  # 32768
R_TOK = 2048                     # tokens per round
N_R = TOK_CORE // R_TOK          # 16 rounds
J = R_TOK // 128                 # 16 [128, 256] tiles per round
JH = J // 2                      # 8 tiles per half-round
NDB = DIM // 32                  # 8 d-blocks of 32
EPS = 1e-5

_BUILD_CACHE = {}


def build_kernel(use_b2c=False, use_gamma=False, use_beta=False,
                 repeat=1, variant="full"):
    """Build the per-core Bass program. Returns the compiled Bacc object."""
    key = (use_b2c, use_gamma, use_beta, repeat, variant)
    if key in _BUILD_CACHE:
        return _BUILD_CACHE[key]

    import concourse.bacc as bacc
    import concourse.mybir as mybir
    from concourse.tile import TileContext

    f32 = mybir.dt.float32
    bf16 = mybir.dt.bfloat16
    AF = mybir.ActivationFunctionType
    ALU = mybir.AluOpType

    nc = bacc.Bacc("TRN2")
    x_d = nc.dram_tensor("x", [TOK_CORE, DIM], f32, kind="ExternalInput")
    # f32 consts: col 0 b1 (replicated per 32-group), 8:16 b2-mean(b2),
    # 16:24 gamma, 24:32 beta
    wp_d = nc.dram_tensor("wpack", [128, 32], f32, kind="ExternalInput")
    # bf16 consts: cols 0:256 w1t blocks, 256:288 w2t9 (replicated per
    # 32-group)
    wb_d = nc.dram_tensor("wpackb", [128, 288], bf16, kind="ExternalInput")
    y_d = nc.dram_tensor("y", [TOK_CORE, OUT], f32, kind="ExternalOutput")

    # token t = r*2048 + p*16 + f: each partition reads one contiguous
    # 16 KB run per round and writes one contiguous 512 B run.
    x_v = x_d[:, :].rearrange("(r p f) d -> r p f d", r=N_R, p=128, f=J)
    y_v = y_d[:, :].rearrange("(r p f) c -> r p f c", r=N_R, p=128, f=J)

    with TileContext(nc) as tc:
        with (
            tc.tile_pool(name="consts", bufs=1) as consts,
            tc.tile_pool(name="xin", bufs=4) as xin,
            tc.tile_pool(name="xcp", bufs=3) as xcp,
            tc.tile_pool(name="xtp", bufs=3) as xtp,
            tc.tile_pool(name="h1p", bufs=3) as h1p,
            tc.tile_pool(name="ytp", bufs=3) as ytp,
            tc.tile_pool(name="sqp", bufs=2) as sqp,
            tc.tile_pool(name="accp", bufs=1) as accp,
            tc.tile_pool(name="yout", bufs=8) as yout,
            tc.tile_pool(name="pp", bufs=2, space="PSUM") as pp,
            tc.tile_pool(name="pp2", bufs=2, space="PSUM") as pp2,
        ):
            wp = consts.tile([128, 32], f32)
            nc.sync.dma_start(out=wp, in_=wp_d[:, :])
            wb = consts.tile([128, 288], bf16)
            nc.sync.dma_start(out=wb, in_=wb_d[:, :])
            w1t = wb[:, 0:DIM]
            w2t = wb[:, DIM : DIM + 32]
            b1c = wp[:, 0:1]
            aux = wp[:, 8:32]
            eps_c = consts.tile([128, 1], f32)
            nc.vector.memset(eps_c, EPS)

            # split accumulators per finalize batch: no shared tile
            # between in-flight rounds and a draining finalize.
            cent_b = [
                accp.tile([128, (N_R // 2) * 128], f32, name=f"cent{b}",
                          tag=f"cent{b}")
                for b in range(2)
            ]
            ssq_b = [
                accp.tile([128, (N_R // 2) * 16], f32, name=f"ssq{b}",
                          tag=f"ssq{b}")
                for b in range(2)
            ]

            def dma_only_pass():
                for r in range(N_R):
                    x_sb = xin.tile([128, J, DIM], f32, tag="x_sb")
                    nc.sync.dma_start(out=x_sb, in_=x_v[r])
                    y_t = yout.tile([128, J, 8], f32, tag="y_t")
                    nc.vector.tensor_copy(out=y_t[:, 0:1, :], in_=x_sb[:, 0:1, 0:8])
                    nc.gpsimd.dma_start(out=y_v[r], in_=y_t)

            def finalize(b):
                # rstd for batch b (rounds b*8 .. b*8+8) + scale + store.
                nr = N_R // 2
                r_lo = b * nr
                stdv = sqp.tile([128, nr * 16], f32, tag="stdv")
                nc.scalar.activation(
                    out=stdv,
                    in_=ssq_b[b],
                    func=AF.Sqrt,
                    bias=eps_c[:, 0:1],
                    scale=1.0 / OUT,
                )
                rstd = sqp.tile([128, nr * 16], f32, tag="rstd")
                nc.vector.reciprocal(out=rstd, in_=stdv)
                for i in range(nr):
                    y_t = yout.tile([128, J, 8], f32, tag="y_t")
                    cent_r = cent_b[b][:, i * 128 : (i + 1) * 128].rearrange(
                        "p (j c) -> p j c", c=8
                    )
                    rs = rstd[:, i * 16 : (i + 1) * 16].rearrange(
                        "p (j c) -> p j c", c=1
                    ).broadcast_to([128, J, 8])
                    nc.gpsimd.tensor_tensor(
                        out=y_t, in0=cent_r, in1=rs, op=ALU.mult
                    )
                    if use_gamma:
                        gm = aux[:, 8:16].rearrange(
                            "p (j c) -> p j c", j=1
                        ).broadcast_to([128, J, 8])
                        nc.vector.tensor_tensor(
                            out=y_t, in0=y_t, in1=gm, op=ALU.mult
                        )
                    if use_beta:
                        bt = aux[:, 16:24].rearrange(
                            "p (j c) -> p j c", j=1
                        ).broadcast_to([128, J, 8])
                        nc.vector.tensor_tensor(
                            out=y_t, in0=y_t, in1=bt, op=ALU.add
                        )
                    nc.gpsimd.dma_start(out=y_v[r_lo + i], in_=y_t)

            def load_x(r):
                # ---- load x rows (token-major) ----
                x_sb = xin.tile([128, J, DIM], f32, tag="x_sb")
                nc.sync.dma_start(out=x_sb, in_=x_v[r])
                return x_sb

            def cast_x(x_sb):
                # ---- downcast to bf16 (StreamTranspose requires same
                # src/dst dtype, so cast first; ACT casts at ~1ns/col,
                # GpSimd at ~3.4ns/col, so ACT gets 3.5 chunks,
                # GpSimd 0.5) ----
                xc = xcp.tile([128, J, DIM], bf16, tag="xc")
                w = J // 4
                for q in range(4):
                    src = x_sb[:, w * q : w * (q + 1), :]
                    dst = xc[:, w * q : w * (q + 1), :]
                    if q < 3:
                        nc.scalar.activation(
                            out=dst, in_=src, func=AF.Copy,
                            bias=0.0, scale=1.0,
                        )
                    else:
                        nc.scalar.activation(
                            out=dst[:, 0 : w // 2, :],
                            in_=src[:, 0 : w // 2, :],
                            func=AF.Copy, bias=0.0, scale=1.0,
                        )
                        nc.gpsimd.tensor_copy(
                            out=dst[:, w // 2 : w, :],
                            in_=src[:, w // 2 : w, :],
                        )
                return xc

            def transpose_x(xc):
                # ---- 32x32 block transpose to feature-major ----
                xt = xtp.tile([128, J, DIM], bf16, tag="xt")
                for q in range(2):
                    w = J // 2
                    nc.vector.transpose(
                        out=xt[:, w * q : w * (q + 1), :],
                        in_=xc[:, w * q : w * (q + 1), :],
                    )
                # xt[32P+a, j, 32*db+b] = x[token r*2048 + j*128 + 32P + b,
                #                           d = 32*db + a]
                return xt

            def one_pass():
              if variant == "dmaonly":
                  dma_only_pass()
                  return
              # software-pipelined with a 1-round skew: loads, casts and
              # transposes for round r+1 are EMITTED before round r's
              # mm1/GELU/mm2, so the in-order ACT/DVE streams never park
              # next-round independent work behind a dependent op.
              xts = {}
              x_sbs = {0: load_x(0), 1: load_x(1), 2: load_x(2)}
              xts[0] = transpose_x(cast_x(x_sbs.pop(0)))
              for r in range(N_R):
                  b, i = divmod(r, N_R // 2)
                  if r + 3 < N_R:
                      x_sbs[r + 3] = load_x(r + 3)
                  if r + 1 < N_R:
                      xts[r + 1] = transpose_x(cast_x(x_sbs.pop(r + 1)))
                  xt_b = xts.pop(r).rearrange("p j (db b) -> p j db b", b=32)

                  # ---- mm1: 4 diagonal streams, one full psum bank,
                  # double-buffered so mm1(r+1) overlaps GELU(r) ----
                  ps = pp.tile([128, 512], f32, name="ps", tag="ps")
                  for db in range(NDB):
                      for P in range(4):
                          nc.tensor.matmul(
                              out=ps[32 * P : 32 * P + 32, :],
                              lhsT=w1t[
                                  32 * P : 32 * P + 32,
                                  32 * db : 32 * db + 32,
                              ],
                              rhs=xt_b[32 * P : 32 * P + 32, :, db, :],
                              start=(db == 0),
                              stop=(db == NDB - 1),
                              tile_position=(32 * P, 32 * P),
                              skip_group_check=True,
                          )

                  # ---- exact GELU (erf) on all 128 lanes, + b1,
                  # bf16 out feeding mm2 ----
                  h1 = h1p.tile([128, 512], bf16, tag="h1")
                  nc.scalar.activation(
                      out=h1, in_=ps, func=AF.Gelu, bias=b1c, scale=1.0
                  )

                  # ---- mm2: 4 diagonal K=8 streams ----
                  ps2 = pp2.tile([128, 512], f32, name="ps2", tag="ps2")
                  for g in range(4):
                      nc.tensor.matmul(
                          out=ps2[32 * g : 32 * g + 32, :],
                          lhsT=w2t[32 * g : 32 * g + 8, 0:32],
                          rhs=h1[32 * g : 32 * g + 8, :],
                          start=True,
                          stop=True,
                          tile_position=(32 * g, 32 * g),
                          skip_group_check=True,
                      )
                  yt = ytp.tile([128, J, 32], f32, tag="yt")
                  nc.vector.transpose(out=yt, in_=ps2[:, :])
                  # yt[p, j, c]: c 0..7 = h2 channels, c 8 = mean

                  cent = cent_b[b][:, i * 128 : (i + 1) * 128].rearrange(
                      "p (j c) -> p j c", c=8
                  )
                  mu = yt[:, :, 8:9].broadcast_to([128, J, 8])
                  nc.gpsimd.tensor_tensor(
                      out=cent, in0=yt[:, :, 0:8], in1=mu, op=ALU.subtract
                  )
                  if use_b2c:
                      b2c = aux[:, 0:8].rearrange(
                          "p (j c) -> p j c", j=1
                      ).broadcast_to([128, J, 8])
                      nc.vector.tensor_tensor(
                          out=cent, in0=cent, in1=b2c, op=ALU.add
                      )

                  # ---- sum of squares per token ----
                  sq = sqp.tile([128, 128], f32, tag="sq")
                  nc.gpsimd.tensor_tensor(
                      out=sq,
                      in0=cent_b[b][:, i * 128 : (i + 1) * 128],
                      in1=cent_b[b][:, i * 128 : (i + 1) * 128],
                      op=ALU.mult,
                  )
                  nc.vector.reduce_sum(
                      out=ssq_b[b][:, i * 16 : (i + 1) * 16],
                      in_=sq.rearrange("p (j c) -> p j c", c=8),
                      axis=mybir.AxisListType.X,
                  )
                  if r == N_R // 2 - 1:
                      finalize(0)
                  elif r == N_R - 1:
                      finalize(1)

            for _rep in range(repeat):
                one_pass()

    nc.compile()
    _BUILD_CACHE[key] = nc
    return nc


def prep_inputs(x, W1, b1, W2, b2, gamma, beta):
    """Host-side prep: shard x, lay out the tiny weights for the kernel."""
    import ml_dtypes

    x = np.ascontiguousarray(np.asarray(x, dtype=np.float32)).reshape(TOK_TOTAL, DIM)
    W1 = np.asarray(W1, dtype=np.float32)
    b1 = np.asarray(b1, dtype=np.float32)
    W2 = np.asarray(W2, dtype=np.float32)
    b2 = np.asarray(b2, dtype=np.float32)
    gamma = np.asarray(gamma, dtype=np.float32)
    beta = np.asarray(beta, dtype=np.float32)

    # w1t[32P+a, 32db+b] = W1[b, 32db+a] (b < 8), replicated per P group
    w1v = W1.reshape(OUT, NDB, 32)                       # [b, db, a]
    w1g = np.zeros((32, NDB, 32), np.float32)            # [a, db, bslot]
    w1g[:, :, :OUT] = np.transpose(w1v, (2, 1, 0))
    w1t = np.tile(w1g.reshape(32, DIM), (4, 1))

    # w2t9[32g+o, m] = W2[m, o] (o < 8); col 8 = mean over rows of W2,
    # replicated into each 32-partition group
    w2t9 = np.zeros((32, 32), np.float32)
    w2t9[:OUT, :OUT] = W2.T
    w2t9[:OUT, 8] = W2.mean(axis=0)
    w2rep = np.tile(w2t9, (4, 1))

    use_b2c = bool(np.any(b2 != 0.0))
    use_gamma = bool(np.any(gamma != 1.0))
    use_beta = bool(np.any(beta != 0.0))

    wpackb = np.zeros((128, 288), ml_dtypes.bfloat16)
    wpackb[:, 0:DIM] = w1t.astype(ml_dtypes.bfloat16)
    wpackb[:, DIM : DIM + 32] = w2rep.astype(ml_dtypes.bfloat16)

    wpack = np.zeros((128, 32), np.float32)
    b1full = np.zeros((128,), np.float32)
    for g in range(4):
        b1full[32 * g : 32 * g + OUT] = b1
    wpack[:, 0] = b1full
    wpack[:, 8:16] = (b2 - b2.mean())[None, :]
    wpack[:, 16:24] = gamma[None, :]
    wpack[:, 24:32] = beta[None, :]

    in_maps = []
    for k in range(N_CORES):
        m = {
            "x": np.ascontiguousarray(x[k * TOK_CORE : (k + 1) * TOK_CORE]),
            "wpack": wpack,
            "wpackb": wpackb,
        }
        in_maps.append(m)
    flags = dict(use_b2c=use_b2c, use_gamma=use_gamma, use_beta=use_beta)
    return in_maps, flags


def run(x, W1, b1, W2, b2, gamma, beta, trace=False, variant="full", **kw):
    from concourse.bass_utils import run_bass_kernel_spmd

    kw.pop("mm_f32r", None)
    in_maps, flags = prep_inputs(x, W1, b1, W2, b2, gamma, beta)
    nc = build_kernel(variant=variant, **flags)
    res = run_bass_kernel_spmd(
        nc, in_maps, core_ids=list(range(N_CORES)), trace=trace, **kw
    )
    y = np.concatenate([res.results[k]["y"] for k in range(N_CORES)], axis=0)
    return y.reshape(B, T, OUT).astype(np.float32), res


def kernel(x, W1, b1, W2, b2, gamma, beta):
    y, _ = run(x, W1, b1, W2, b2, gamma, beta)
    return y
